# revision 27
# baseline (speedup 1.0000x reference)
"""Trainium2 Bass kernel: custom multi-head attention (seq-first, packed in-proj,
"faithful" non-standard head recombine) sharded over 8 NeuronCores.

Sharding: batch*heads across cores. B=2, H=16 -> 32 (b,h) pairs, 4 per core
(same batch per core). Each core computes, for its 4 heads:
  QKV projection (tensor-parallel column slice of in_proj), full-seq attention,
  and the final out-projection rows for its heads (the reference's
  out.reshape(B,T,E) maps head h's (T,D) block to output rows h*128..h*128+127,
  so each core produces a disjoint slice of the output -> no collectives).

All matmul operands fp16 (fp32 PSUM accumulation). Softmax without max-
subtraction (logits are O(5), exp is safe in fp32), denominators via
ones-column matmuls, normalization by reciprocal + DMA partition-broadcast.

Layout core ideas:
 - x^T (E,T) prepared on host per batch; projections produce Q^T/K^T/V^T with
   two heads packed on partition halves (pair A: partitions 0-63, B: 64-127).
 - S^T = K·Q^T computed with 2 pairs row-packed in the PE array (K=64 each).
 - exp runs on ACT straight out of PSUM ([128,1024] activations), scale=D^-0.5.
 - P@V col-packed (M=64 per pair); denominators via ones-column matmuls.
 - Final projection consumes the "scrambled reshape" analytically:
   Y[r,f] = sum_j out^T[:, j::16].T @ out_w[j*64:(j+1)*64, f*...] with j=0..15,
   row-packed across the two pairs (needs out_w in natural + half-swapped
   copies, prepared on host).
"""

import numpy as np
from contextlib import ExitStack

import concourse.bass as bass
import concourse.mybir as mybir
import concourse.tile as tile
from concourse import bacc
from concourse.bass_utils import run_bass_kernel_spmd
from concourse.masks import make_identity

F16 = mybir.dt.float16
F32 = mybir.dt.float32

B, E, H, D = 2, 1024, 16, 64
NCORES = 8
SCALE = D ** -0.5
P = 128


def build_nc(T=2048, debug=False):
    """Build + compile the SPMD single-core program (same on all 8 cores)."""
    n_ks = T // P            # key slices of 128
    n_sup = n_ks // 2        # exp super-tiles (2 key-slices each)
    n_qt = max(T // 512, 1)  # query tiles of 512
    QW = min(T, 512)         # query tile width
    n_es = E // P            # 8 contraction slices for projections
    n_tt = T // 512 if T >= 512 else 1  # t-tiles for projections
    TW = min(T, 512)

    nc = bacc.Bacc("TRN2", target_bir_lowering=False, debug=False)

    xT = nc.dram_tensor("xT", [E, T], F16, kind="ExternalInput").ap()
    wqkv = nc.dram_tensor("wqkv", [2, 3, E, 128], F16, kind="ExternalInput").ap()
    bqkv = nc.dram_tensor("bqkv", [2, 3, 128], F32, kind="ExternalInput").ap()
    wo2 = nc.dram_tensor("wo2", [2, 8, 128, E], F16, kind="ExternalInput").ap()
    ob = nc.dram_tensor("ob", [E], F16, kind="ExternalInput").ap()
    y = nc.dram_tensor("y", [4, T // 16, E], F32, kind="ExternalOutput").ap()
    if debug:
        dbg_qkv = nc.dram_tensor(
            "dbg_qkv", [3, P, 2, T], F16, kind="ExternalOutput"
        ).ap()
        dbg_outT = nc.dram_tensor(
            "dbg_outT", [P, 2, T], F16, kind="ExternalOutput"
        ).ap()
        dbg_vnat = nc.dram_tensor(
            "dbg_vnat", [P, 2, 2, T // P, D], F16, kind="ExternalOutput"
        ).ap()
        dbg_dn = nc.dram_tensor(
            "dbg_dn", [2, 2, T // 512 if T >= 512 else 1, 512],
            F32, kind="ExternalOutput"
        ).ap()

    with tile.TileContext(nc) as tc, ExitStack() as ctx:
        consts = ctx.enter_context(tc.tile_pool(name="consts", bufs=1))
        sb_w = ctx.enter_context(tc.tile_pool(name="sb_w", bufs=1))
        sb_x = ctx.enter_context(tc.tile_pool(name="sb_x", bufs=1))
        sb_qkv = ctx.enter_context(tc.tile_pool(name="sb_qkv", bufs=1))
        sb_wo = ctx.enter_context(tc.tile_pool(name="sb_wo", bufs=1))
        ring = ctx.enter_context(tc.tile_pool(name="ring", bufs=3))
        norm = ctx.enter_context(tc.tile_pool(name="norm", bufs=2))

        # ---- constants ----
        ones_t = consts.tile([P, P], F16, tag="ones")
        nc.vector.memset(ones_t[:], 1.0)
        ident = consts.tile([P, P], F16, tag="ident")
        make_identity(nc, ident[:])
        ob_sb = consts.tile([1, E], F16, tag="ob")
        nc.sync.dma_start(ob_sb[:], ob.unsqueeze(0))
        bias_sb = consts.tile([P, 2, 3], F32, tag="bias")
        for g in range(2):
            for t in range(3):
                nc.sync.dma_start(
                    bias_sb[:, g, t].unsqueeze(1), bqkv[g, t].unsqueeze(1)
                )

        # ---- weights / x loads ----
        w_sb = sb_w.tile([P, 2, 3, n_es, P], F16, tag="wqkv")
        nc.sync.dma_start(
            w_sb[:], wqkv.rearrange("g t (es p) c -> p g t es c", p=P)
        )
        x_sb = sb_x.tile([P, n_es, T], F16, tag="xT")
        nc.sync.dma_start(x_sb[:], xT.rearrange("(es p) t -> p es t", p=P))
        wo_sb = sb_wo.tile([P, 2, 8, E], F16, tag="wo")
        nc.sync.dma_start(wo_sb[:], wo2.rearrange("c s p f -> p c s f"))

        qt_sb = sb_qkv.tile([P, 2, T], F16, tag="QT")
        kt_sb = sb_qkv.tile([P, 2, T], F16, tag="KT")
        vt_sb = sb_qkv.tile([P, 2, T], F16, tag="VT")
        vnat = sb_qkv.tile([P, 2, 2, n_ks, D], F16, tag="Vnat")
        outT = sb_qkv.tile([P, 2, T], F16, tag="outT")

        dest_of = {0: qt_sb, 1: kt_sb, 2: vt_sb}

        def proj(g, t, ps_acc):
            # projection type t (0=q,1=k,2=v) for head-pair group g
            for tt in range(n_tt):
                ps = ps_acc.tile([P, TW], F32, tag="op_a" if tt % 2 == 0 else "op_b")
                for es in range(n_es):
                    nc.tensor.matmul(
                        ps[:],
                        lhsT=w_sb[:, g, t, es, :],
                        rhs=x_sb[:, es, tt * TW:(tt + 1) * TW],
                        start=(es == 0),
                        stop=(es == n_es - 1),
                    )
                nc.vector.tensor_scalar_add(
                    dest_of[t][:, g, tt * TW:(tt + 1) * TW],
                    ps[:],
                    bias_sb[:, g, t].unsqueeze(1),
                )

        def v_transpose(g, ps_st):
            # V^T (d,t) -> V (t,d), 8 key-slices batched per PSUM bank
            nb = max(n_ks // 8, 1)
            for pr in range(2):
                lo = pr * 64
                for bi in range(nb):
                    nks_b = min(8, n_ks - bi * 8)
                    trp = ps_st.tile([P, 8 * D], F16, tag="st_a")
                    for kk in range(nks_b):
                        ks = bi * 8 + kk
                        nc.tensor.transpose(
                            trp[:, kk * D:(kk + 1) * D],
                            in_=vt_sb[lo:lo + 64, g, ks * P:(ks + 1) * P],
                            identity=ident[lo:lo + 64, lo:lo + 64],
                            tile_position=(lo, 0),
                        )
                    nc.any.tensor_copy(
                        vnat[:, g, pr, bi * 8:bi * 8 + nks_b, :],
                        trp[:, :nks_b * D],
                    )

        def attention(g, ps_st, ps_acc, ps_dn):
            for qt in range(n_qt):
                q0 = qt * QW
                st_a = ps_st.tile([P, 2 * QW], F32, tag="st_a")
                st_b = ps_st.tile([P, 2 * QW], F32, tag="st_b")
                op_a = ps_acc.tile([P, QW], F32, tag="op_a")
                op_b = ps_acc.tile([P, QW], F32, tag="op_b")
                dn_a = ps_dn.tile([33, QW], F32, tag="dn_a")
                dn_b = ps_dn.tile([33, QW], F32, tag="dn_b")
                for s in range(n_sup):
                    pt_a = ring.tile([P, 2 * QW], F16, tag="pt_a")
                    pt_b = ring.tile([P, 2 * QW], F16, tag="pt_b")
                    for sub in range(2):
                        ks = 2 * s + sub
                        nc.tensor.matmul(
                            st_a[:, sub * QW:(sub + 1) * QW],
                            lhsT=kt_sb[0:64, g, ks * P:(ks + 1) * P],
                            rhs=qt_sb[0:64, g, q0:q0 + QW],
                            start=True, stop=True,
                            tile_position=(0, 0),
                        )
                        nc.tensor.matmul(
                            st_b[:, sub * QW:(sub + 1) * QW],
                            lhsT=kt_sb[64:128, g, ks * P:(ks + 1) * P],
                            rhs=qt_sb[64:128, g, q0:q0 + QW],
                            start=True, stop=True,
                            tile_position=(64, 0),
                        )
                    nc.scalar.activation(
                        pt_a[:], st_a[:], mybir.ActivationFunctionType.Exp,
                        scale=SCALE,
                    )
                    nc.scalar.activation(
                        pt_b[:], st_b[:], mybir.ActivationFunctionType.Exp,
                        scale=SCALE,
                    )
                    for sub in range(2):
                        ks = 2 * s + sub
                        first, last = (ks == 0), (ks == n_ks - 1)
                        nc.tensor.matmul(
                            op_a[0:64, :],
                            lhsT=vnat[:, g, 0, ks, :],
                            rhs=pt_a[:, sub * QW:(sub + 1) * QW],
                            start=first, stop=last,
                            tile_position=(0, 0),
                        )
                        nc.tensor.matmul(
                            op_b[64:128, :],
                            lhsT=vnat[:, g, 1, ks, :],
                            rhs=pt_b[:, sub * QW:(sub + 1) * QW],
                            start=first, stop=last,
                            tile_position=(0, 64),
                        )
                        nc.tensor.matmul(
                            dn_a[0:1, :],
                            lhsT=ones_t[:, 0:1],
                            rhs=pt_a[:, sub * QW:(sub + 1) * QW],
                            start=first, stop=last,
                            tile_position=(0, 0),
                        )
                        nc.tensor.matmul(
                            dn_b[32:33, :],
                            lhsT=ones_t[:, 0:1],
                            rhs=pt_b[:, sub * QW:(sub + 1) * QW],
                            start=first, stop=last,
                            tile_position=(0, 32),
                        )
                # normalization: out^T = op / denom  (denom broadcast over d
                # via gpsimd partition_broadcast; gpsimd is otherwise idle)
                rcp = norm.tile([33, QW], F32, tag="rcp")
                nc.vector.reciprocal(rcp[0:1, :], dn_a[0:1, :])
                nc.vector.reciprocal(rcp[32:33, :], dn_b[32:33, :])
                if debug:
                    nc.sync.dma_start(
                        dbg_dn[g, 0, qt].unsqueeze(0), rcp[0:1, :]
                    )
                    nc.sync.dma_start(
                        dbg_dn[g, 1, qt].unsqueeze(0), rcp[32:33, :]
                    )
                rbs = norm.tile([P, QW], F32, tag="rbs")
                rcp16 = norm.tile([33, QW], F16, tag="rcp16")
                nc.any.tensor_copy(rcp16[0:1, :], rcp[0:1, :])
                nc.any.tensor_copy(rcp16[32:33, :], rcp[32:33, :])
                rbs_pa = ps_dn.tile([P, QW], F32, tag="dn_a")
                rbs_pb = ps_dn.tile([P, QW], F32, tag="dn_b")
                nc.tensor.matmul(
                    rbs_pa[0:64, :], lhsT=ones_t[0:1, 0:64],
                    rhs=rcp16[0:1, :],
                    start=True, stop=True, tile_position=(0, 0),
                )
                nc.tensor.matmul(
                    rbs_pb[64:128, :], lhsT=ones_t[32:33, 0:64],
                    rhs=rcp16[32:33, :],
                    start=True, stop=True, tile_position=(32, 64),
                )
                nc.any.tensor_copy(rbs[0:64, :], rbs_pa[0:64, :])
                nc.any.tensor_copy(rbs[64:128, :], rbs_pb[64:128, :])
                nc.vector.tensor_mul(
                    outT[0:64, g, q0:q0 + QW], op_a[0:64, :], rbs[0:64, :]
                )
                nc.vector.tensor_mul(
                    outT[64:128, g, q0:q0 + QW], op_b[64:128, :],
                    rbs[64:128, :],
                )

        R = T // 16  # output rows per head block

        def out_proj(g, ps_acc):
            # Y[r,f] = sum_j outT[:, j::16].T @ W_j + ob ; row-packed pairs
            oT = outT[:, g, :].rearrange("p (r j) -> p j r", j=16)
            for ft in range(E // 512):
                f0 = ft * 512
                yp_a = ps_acc.tile([P, 512], F32, tag="op_a")
                yp_b = ps_acc.tile([P, 512], F32, tag="op_b")
                for j in range(16):
                    s = j // 2
                    ca = 0 if j % 2 == 0 else 1
                    cb = 1 - ca
                    nc.tensor.matmul(
                        yp_a[0:R, :],
                        lhsT=oT[0:64, j, :],
                        rhs=wo_sb[0:64, ca, s, f0:f0 + 512],
                        start=(j == 0), stop=False,
                        tile_position=(0, 0),
                    )
                    nc.tensor.matmul(
                        yp_b[0:R, :],
                        lhsT=oT[64:128, j, :],
                        rhs=wo_sb[64:128, cb, s, f0:f0 + 512],
                        start=(j == 0), stop=False,
                        tile_position=(64, 0),
                    )
                nc.tensor.matmul(
                    yp_a[0:R, :], lhsT=ones_t[0:1, 0:R],
                    rhs=ob_sb[0:1, f0:f0 + 512],
                    start=False, stop=True, tile_position=(0, 0),
                )
                nc.tensor.matmul(
                    yp_b[0:R, :], lhsT=ones_t[0:1, 0:R],
                    rhs=ob_sb[0:1, f0:f0 + 512],
                    start=False, stop=True, tile_position=(0, 0),
                )
                ys_a = norm.tile([P, 512], F32, tag="ystage")
                ys_b = norm.tile([P, 512], F32, tag="ystage")
                nc.any.tensor_copy(ys_a[0:R, :], yp_a[0:R, :])
                nc.any.tensor_copy(ys_b[0:R, :], yp_b[0:R, :])
                nc.sync.dma_start(y[2 * g + 0, :, f0:f0 + 512], ys_a[0:R, :])
                nc.sync.dma_start(y[2 * g + 1, :, f0:f0 + 512], ys_b[0:R, :])

        # ---- phase emission (g-major for ACT continuity) ----
        # PSUM budget (8 banks): st_a 2 + st_b 2 + op_a 1 + op_b 1 +
        # dn_a 1 + dn_b 1; v-transposes and projections share these tags.
        with tc.tile_pool(name="ps_st", bufs=1, space="PSUM") as ps_st, \
             tc.tile_pool(name="ps_acc", bufs=1, space="PSUM") as ps_acc, \
             tc.tile_pool(name="ps_dn", bufs=1, space="PSUM") as ps_dn:
            for g in range(2):
                proj(g, 1, ps_acc)   # K first: S needs all of K^T
                proj(g, 0, ps_acc)   # Q
                proj(g, 2, ps_acc)   # V
                v_transpose(g, ps_st)
                attention(g, ps_st, ps_acc, ps_dn)
                out_proj(g, ps_acc)
            if debug:
                nc.sync.dma_start(dbg_qkv[0], qt_sb[:])
                nc.sync.dma_start(dbg_qkv[1], kt_sb[:])
                nc.sync.dma_start(dbg_qkv[2], vt_sb[:])
                nc.sync.dma_start(dbg_outT[:], outT[:])
                nc.sync.dma_start(dbg_vnat[:], vnat[:])

    nc.compile()
    return nc


_NC_CACHE = {}


def _get_nc(T=2048):
    if T not in _NC_CACHE:
        _NC_CACHE[T] = build_nc(T)
    return _NC_CACHE[T]


def make_in_maps(x, in_proj_w, in_proj_b, out_w, out_b):
    """Host-side sharding/layout prep -> per-core input maps."""
    T, Bx, Ex = x.shape
    x = np.asarray(x, np.float32)
    in_proj_w = np.asarray(in_proj_w, np.float32)
    in_proj_b = np.asarray(in_proj_b, np.float32)
    out_w = np.asarray(out_w, np.float32)
    out_b = np.asarray(out_b, np.float32)

    # out_w natural + half-swapped copies, pre-tiled to [2, 8, 128, E]
    wo_nat = out_w.reshape(8, 128, Ex)
    wo_swp = np.concatenate(
        [wo_nat[:, 64:128, :], wo_nat[:, 0:64, :]], axis=1
    )
    wo2 = np.stack([wo_nat, wo_swp], axis=0).astype(np.float16)
    ob = out_b.astype(np.float16)

    in_maps = []
    for c in range(NCORES):
        b = c // 4
        h0 = 4 * (c % 4)
        xT = np.ascontiguousarray(x[:, b, :].T).astype(np.float16)
        wq, bq = [], []
        for g in range(2):
            ha = h0 + 2 * g
            cols = slice(ha * D, ha * D + 128)
            wq.append(np.stack(
                [in_proj_w[:, cols],
                 in_proj_w[:, Ex + ha * D: Ex + ha * D + 128],
                 in_proj_w[:, 2 * Ex + ha * D: 2 * Ex + ha * D + 128]], axis=0))
            bq.append(np.stack(
                [in_proj_b[cols],
                 in_proj_b[Ex + ha * D: Ex + ha * D + 128],
                 in_proj_b[2 * Ex + ha * D: 2 * Ex + ha * D + 128]], axis=0))
        in_maps.append({
            "xT": xT,
            "wqkv": np.stack(wq, axis=0).astype(np.float16),
            "bqkv": np.stack(bq, axis=0).astype(np.float32),
            "wo2": wo2,
            "ob": ob,
        })
    return in_maps


def assemble(results, T, Ex):
    R = T // 16
    yf = np.empty((B, T, Ex), np.float32)
    for c in range(NCORES):
        b = c // 4
        h0 = 4 * (c % 4)
        blk = results[c]["y"]  # [4, T//16, E]
        for i in range(4):
            h = h0 + i
            yf[b, h * R:(h + 1) * R, :] = blk[i]
    return yf


def kernel(x, in_proj_w, in_proj_b, out_w, out_b):
    T = x.shape[0]
    nc = _get_nc(T)
    in_maps = make_in_maps(x, in_proj_w, in_proj_b, out_w, out_b)
    res = run_bass_kernel_spmd(nc, in_maps, core_ids=list(range(NCORES)))
    return assemble(res.results, T, x.shape[2])


# revision 33
# speedup vs baseline: 16.6099x; 16.6099x over previous
"""Trainium2 Bass kernel: custom multi-head attention (seq-first, packed in-proj,
"faithful" non-standard head recombine) sharded over 8 NeuronCores.

Sharding: batch*heads across cores. B=2, H=16 -> 32 (b,h) pairs, 4 per core
(same batch per core). Each core computes, for its 4 heads:
  QKV projection (tensor-parallel column slice of in_proj), full-seq attention,
  and the final out-projection rows for its heads (the reference's
  out.reshape(B,T,E) maps head h's (T,D) block to output rows h*128..h*128+127,
  so each core produces a disjoint slice of the output -> no collectives).

All matmul operands fp16 (fp32 PSUM accumulation). Softmax without max-
subtraction (logits are O(5), exp is safe in fp32), denominators via
ones-column matmuls, normalization by reciprocal + rank-1 PE broadcast.

Layout core ideas:
 - x^T (E,T) prepared on host per batch; projections produce Q^T/K^T/V^T with
   two heads packed on partition halves (pair A: partitions 0-63, B: 64-127).
 - S^T = K·Q^T computed with 2 pairs row-packed in the PE array (K=64 each).
 - exp runs on ACT straight out of PSUM ([128,1024] activations), scale=D^-0.5.
 - P@V col-packed (M=64 per pair); denominators via ones-column matmuls.
 - Final projection consumes the "scrambled reshape" analytically:
   Y[r,f] = sum_j out^T[:, j::16].T @ out_w[j*64:(j+1)*64, f*...] with j=0..15,
   row-packed across the two pairs (needs out_w in natural + half-swapped
   copies, prepared on host).
"""

import numpy as np
from contextlib import ExitStack

import concourse.bass as bass
import concourse.mybir as mybir
import concourse.tile as tile
from concourse import bacc
from concourse.bass_utils import run_bass_kernel_spmd
from concourse.masks import make_identity

F16 = mybir.dt.float16
F32 = mybir.dt.float32

B, E, H, D = 2, 1024, 16, 64
NCORES = 8
SCALE = D ** -0.5
P = 128


def build_nc(T=2048, debug=False, reps=1):
    """Build + compile the SPMD single-core program (same on all 8 cores).

    reps>1 replicates the whole compute body (same inputs/outputs) for
    on-hardware timing via the slope between reps variants.
    """
    n_ks = T // P            # key slices of 128
    n_sup = n_ks // 2        # exp super-tiles (2 key-slices each)
    n_qt = max(T // 512, 1)  # query tiles of 512
    QW = min(T, 512)         # query tile width
    n_es = E // P            # 8 contraction slices for projections
    n_tt = T // 512 if T >= 512 else 1  # t-tiles for projections
    TW = min(T, 512)

    nc = bacc.Bacc("TRN2", target_bir_lowering=False, debug=False)

    xT = nc.dram_tensor("xT", [E, T], F16, kind="ExternalInput").ap()
    wqkv = nc.dram_tensor("wqkv", [2, 3, E, 128], F16, kind="ExternalInput").ap()
    bqkv = nc.dram_tensor("bqkv", [2, 3, 128], F32, kind="ExternalInput").ap()
    wo2 = nc.dram_tensor("wo2", [2, 8, 128, E], F16, kind="ExternalInput").ap()
    ob = nc.dram_tensor("ob", [E], F16, kind="ExternalInput").ap()
    y = nc.dram_tensor("y", [4, T // 16, E], F32, kind="ExternalOutput").ap()
    if debug:
        dbg_qkv = nc.dram_tensor(
            "dbg_qkv", [3, P, 2, T], F16, kind="ExternalOutput"
        ).ap()
        dbg_outT = nc.dram_tensor(
            "dbg_outT", [P, 2, T], F16, kind="ExternalOutput"
        ).ap()
        dbg_vnat = nc.dram_tensor(
            "dbg_vnat", [P, 2, 2, T // P, D], F16, kind="ExternalOutput"
        ).ap()
        dbg_dn = nc.dram_tensor(
            "dbg_dn", [2, 2, T // 512 if T >= 512 else 1, 512],
            F32, kind="ExternalOutput"
        ).ap()

    with tile.TileContext(nc) as tc, ExitStack() as ctx:
        consts = ctx.enter_context(tc.tile_pool(name="consts", bufs=1))
        sb_w = ctx.enter_context(tc.tile_pool(name="sb_w", bufs=1))
        sb_x = ctx.enter_context(tc.tile_pool(name="sb_x", bufs=1))
        sb_qkv = ctx.enter_context(tc.tile_pool(name="sb_qkv", bufs=1))
        sb_wo = ctx.enter_context(tc.tile_pool(name="sb_wo", bufs=1))
        ring = ctx.enter_context(tc.tile_pool(name="ring", bufs=4))
        norm = ctx.enter_context(tc.tile_pool(name="norm", bufs=2))

        # ---- constants ----
        ones_t = consts.tile([P, P], F16, tag="ones")
        nc.vector.memset(ones_t[:], 1.0)
        ident = consts.tile([P, P], F16, tag="ident")
        make_identity(nc, ident[:])
        ob_sb = consts.tile([1, E], F16, tag="ob")
        nc.sync.dma_start(ob_sb[:], ob.unsqueeze(0))
        bias_sb = consts.tile([P, 2, 3], F32, tag="bias")
        for g in range(2):
            for t in range(3):
                nc.sync.dma_start(
                    bias_sb[:, g, t].unsqueeze(1), bqkv[g, t].unsqueeze(1)
                )

        # ---- weights / x loads (split so the first matmuls start early) ----
        w_sb = sb_w.tile([P, 2, 3, n_es, P], F16, tag="wqkv")
        wq_r = wqkv.rearrange("g t (es p) c -> p g t es c", p=P)
        for g in range(2):
            for t in (1, 0, 2):  # match proj emission order (K, Q, V)
                nc.sync.dma_start(w_sb[:, g, t], wq_r[:, g, t])
        x_sb = sb_x.tile([P, n_es, T], F16, tag="xT")
        xr = xT.rearrange("(es p) t -> p es t", p=P)
        for es in range(n_es):
            nc.sync.dma_start(x_sb[:, es], xr[:, es])
        wo_sb = sb_wo.tile([P, 2, 8, E], F16, tag="wo")
        nc.sync.dma_start(wo_sb[:], wo2.rearrange("c s p f -> p c s f"))

        qt_sb = sb_qkv.tile([P, 2, T], F16, tag="QT")
        kt_sb = sb_qkv.tile([P, 2, T], F16, tag="KT")
        vt_sb = sb_qkv.tile([P, 2, T], F16, tag="VT")
        vnat = sb_qkv.tile([P, 2, 2, n_ks, D], F16, tag="Vnat")
        outT = sb_qkv.tile([P, 2, T], F16, tag="outT")

        dest_of = {0: qt_sb, 1: kt_sb, 2: vt_sb}

        def proj(g, t, ps_acc):
            # projection type t (0=q,1=k,2=v) for head-pair group g
            for tt in range(n_tt):
                ps = ps_acc.tile([P, TW], F32, tag="op_a" if tt % 2 == 0 else "op_b")
                for es in range(n_es):
                    nc.tensor.matmul(
                        ps[:],
                        lhsT=w_sb[:, g, t, es, :],
                        rhs=x_sb[:, es, tt * TW:(tt + 1) * TW],
                        start=(es == 0),
                        stop=(es == n_es - 1),
                    )
                nc.vector.tensor_scalar_add(
                    dest_of[t][:, g, tt * TW:(tt + 1) * TW],
                    ps[:],
                    bias_sb[:, g, t].unsqueeze(1),
                )

        def v_transpose(g, ps_st):
            # V^T (d,t) -> V (t,d), 8 key-slices batched per PSUM bank
            nb = max(n_ks // 8, 1)
            for pr in range(2):
                lo = pr * 64
                for bi in range(nb):
                    nks_b = min(8, n_ks - bi * 8)
                    trp = ps_st.tile([P, 8 * D], F16, tag="st_a")
                    for kk in range(nks_b):
                        ks = bi * 8 + kk
                        nc.tensor.transpose(
                            trp[:, kk * D:(kk + 1) * D],
                            in_=vt_sb[lo:lo + 64, g, ks * P:(ks + 1) * P],
                            identity=ident[lo:lo + 64, lo:lo + 64],
                            tile_position=(lo, 0),
                        )
                    nc.vector.tensor_copy(
                        vnat[:, g, pr, bi * 8:bi * 8 + nks_b, :],
                        trp[:, :nks_b * D],
                    )

        def attention(g, ps_st, ps_acc, ps_dn):
            for qt in range(n_qt):
                q0 = qt * QW
                st_a = ps_st.tile([P, 2 * QW], F32, tag="st_a")
                st_b = ps_st.tile([P, 2 * QW], F32, tag="st_b")
                op_a = ps_acc.tile([P, QW], F32, tag="op_a")
                op_b = ps_acc.tile([P, QW], F32, tag="op_b")
                dn_a = ps_dn.tile([33, QW], F32, tag="dn_a")
                dn_b = ps_dn.tile([33, QW], F32, tag="dn_b")
                for s in range(n_sup):
                    pt_a = ring.tile([P, 2 * QW], F16, tag="pt_a")
                    pt_b = ring.tile([P, 2 * QW], F16, tag="pt_b")
                    for sub in range(2):
                        ks = 2 * s + sub
                        nc.tensor.matmul(
                            st_a[:, sub * QW:(sub + 1) * QW],
                            lhsT=kt_sb[0:64, g, ks * P:(ks + 1) * P],
                            rhs=qt_sb[0:64, g, q0:q0 + QW],
                            start=True, stop=True,
                            tile_position=(0, 0),
                        )
                        nc.tensor.matmul(
                            st_b[:, sub * QW:(sub + 1) * QW],
                            lhsT=kt_sb[64:128, g, ks * P:(ks + 1) * P],
                            rhs=qt_sb[64:128, g, q0:q0 + QW],
                            start=True, stop=True,
                            tile_position=(64, 0),
                        )
                    nc.scalar.activation(
                        pt_a[:], st_a[:], mybir.ActivationFunctionType.Exp,
                        scale=SCALE,
                    )
                    nc.scalar.activation(
                        pt_b[:], st_b[:], mybir.ActivationFunctionType.Exp,
                        scale=SCALE,
                    )
                    for sub in range(2):
                        ks = 2 * s + sub
                        first, last = (ks == 0), (ks == n_ks - 1)
                        nc.tensor.matmul(
                            op_a[0:64, :],
                            lhsT=vnat[:, g, 0, ks, :],
                            rhs=pt_a[:, sub * QW:(sub + 1) * QW],
                            start=first, stop=last,
                            tile_position=(0, 0),
                        )
                        nc.tensor.matmul(
                            op_b[64:128, :],
                            lhsT=vnat[:, g, 1, ks, :],
                            rhs=pt_b[:, sub * QW:(sub + 1) * QW],
                            start=first, stop=last,
                            tile_position=(0, 64),
                        )
                        nc.tensor.matmul(
                            dn_a[0:1, :],
                            lhsT=ones_t[:, 0:1],
                            rhs=pt_a[:, sub * QW:(sub + 1) * QW],
                            start=first, stop=last,
                            tile_position=(0, 0),
                        )
                        nc.tensor.matmul(
                            dn_b[32:33, :],
                            lhsT=ones_t[:, 0:1],
                            rhs=pt_b[:, sub * QW:(sub + 1) * QW],
                            start=first, stop=last,
                            tile_position=(0, 32),
                        )
                # normalization: out^T = op / denom. Copy accumulators to
                # SBUF immediately (frees the op banks for the next q-tile),
                # recip the denominators (frees the dn banks), broadcast the
                # reciprocals over d via rank-1 PE matmuls into the freed dn
                # banks, then multiply (SBUF x PSUM -> SBUF fp16).
                ops = norm.tile([P, QW], F32, tag="ops")
                nc.vector.tensor_copy(ops[0:64, :], op_a[0:64, :])
                nc.vector.tensor_copy(ops[64:128, :], op_b[64:128, :])
                rcp = norm.tile([33, QW], F32, tag="rcp")
                nc.vector.reciprocal(rcp[0:1, :], dn_a[0:1, :])
                nc.vector.reciprocal(rcp[32:33, :], dn_b[32:33, :])
                if debug:
                    nc.sync.dma_start(
                        dbg_dn[g, 0, qt].unsqueeze(0), rcp[0:1, :]
                    )
                    nc.sync.dma_start(
                        dbg_dn[g, 1, qt].unsqueeze(0), rcp[32:33, :]
                    )
                rcp16 = norm.tile([33, QW], F16, tag="rcp16")
                nc.vector.tensor_copy(rcp16[0:1, :], rcp[0:1, :])
                nc.vector.tensor_copy(rcp16[32:33, :], rcp[32:33, :])
                rbs_pa = ps_dn.tile([P, QW], F32, tag="dn_a")
                rbs_pb = ps_dn.tile([P, QW], F32, tag="dn_b")
                nc.tensor.matmul(
                    rbs_pa[0:64, :], lhsT=ones_t[0:1, 0:64],
                    rhs=rcp16[0:1, :],
                    start=True, stop=True, tile_position=(0, 0),
                )
                nc.tensor.matmul(
                    rbs_pb[64:128, :], lhsT=ones_t[32:33, 0:64],
                    rhs=rcp16[32:33, :],
                    start=True, stop=True, tile_position=(32, 64),
                )
                nc.vector.tensor_mul(
                    outT[0:64, g, q0:q0 + QW], ops[0:64, :], rbs_pa[0:64, :]
                )
                nc.vector.tensor_mul(
                    outT[64:128, g, q0:q0 + QW], ops[64:128, :],
                    rbs_pb[64:128, :],
                )

        R = T // 16  # output rows per head block

        def out_proj(g, ps_acc):
            # Y[r,f] = sum_j outT[:, j::16].T @ W_j + ob ; row-packed pairs
            oT = outT[:, g, :].rearrange("p (r j) -> p j r", j=16)
            for ft in range(E // 512):
                f0 = ft * 512
                yp_a = ps_acc.tile([P, 512], F32, tag="op_a")
                yp_b = ps_acc.tile([P, 512], F32, tag="op_b")
                for j in range(16):
                    s = j // 2
                    ca = 0 if j % 2 == 0 else 1
                    cb = 1 - ca
                    nc.tensor.matmul(
                        yp_a[0:R, :],
                        lhsT=oT[0:64, j, :],
                        rhs=wo_sb[0:64, ca, s, f0:f0 + 512],
                        start=(j == 0), stop=False,
                        tile_position=(0, 0),
                    )
                    nc.tensor.matmul(
                        yp_b[0:R, :],
                        lhsT=oT[64:128, j, :],
                        rhs=wo_sb[64:128, cb, s, f0:f0 + 512],
                        start=(j == 0), stop=False,
                        tile_position=(64, 0),
                    )
                nc.tensor.matmul(
                    yp_a[0:R, :], lhsT=ones_t[0:1, 0:R],
                    rhs=ob_sb[0:1, f0:f0 + 512],
                    start=False, stop=True, tile_position=(0, 0),
                )
                nc.tensor.matmul(
                    yp_b[0:R, :], lhsT=ones_t[0:1, 0:R],
                    rhs=ob_sb[0:1, f0:f0 + 512],
                    start=False, stop=True, tile_position=(0, 0),
                )
                ys_a = norm.tile([P, 512], F32, tag="ystage")
                ys_b = norm.tile([P, 512], F32, tag="ystage")
                nc.vector.tensor_copy(ys_a[0:R, :], yp_a[0:R, :])
                nc.vector.tensor_copy(ys_b[0:R, :], yp_b[0:R, :])
                nc.sync.dma_start(y[2 * g + 0, :, f0:f0 + 512], ys_a[0:R, :])
                nc.sync.dma_start(y[2 * g + 1, :, f0:f0 + 512], ys_b[0:R, :])

        # ---- phase emission (g-major for ACT continuity) ----
        # PSUM budget (8 banks): st_a 2 + st_b 2 + op_a 1 + op_b 1 +
        # dn_a 1 + dn_b 1; v-transposes and projections share these tags.
        with tc.tile_pool(name="ps_st", bufs=1, space="PSUM") as ps_st, \
             tc.tile_pool(name="ps_acc", bufs=1, space="PSUM") as ps_acc, \
             tc.tile_pool(name="ps_dn", bufs=1, space="PSUM") as ps_dn:
            for _rep in range(reps):
                for g in range(2):
                    proj(g, 1, ps_acc)   # K first: S needs all of K^T
                    proj(g, 0, ps_acc)   # Q
                    proj(g, 2, ps_acc)   # V
                    v_transpose(g, ps_st)
                    attention(g, ps_st, ps_acc, ps_dn)
                    out_proj(g, ps_acc)
            if debug:
                nc.sync.dma_start(dbg_qkv[0], qt_sb[:])
                nc.sync.dma_start(dbg_qkv[1], kt_sb[:])
                nc.sync.dma_start(dbg_qkv[2], vt_sb[:])
                nc.sync.dma_start(dbg_outT[:], outT[:])
                nc.sync.dma_start(dbg_vnat[:], vnat[:])

    nc.compile()
    return nc


_NC_CACHE = {}


def _get_nc(T=2048):
    if T not in _NC_CACHE:
        _NC_CACHE[T] = build_nc(T)
    return _NC_CACHE[T]


def make_in_maps(x, in_proj_w, in_proj_b, out_w, out_b):
    """Host-side sharding/layout prep -> per-core input maps."""
    T, Bx, Ex = x.shape
    x = np.asarray(x, np.float32)
    in_proj_w = np.asarray(in_proj_w, np.float32)
    in_proj_b = np.asarray(in_proj_b, np.float32)
    out_w = np.asarray(out_w, np.float32)
    out_b = np.asarray(out_b, np.float32)

    # out_w natural + half-swapped copies, pre-tiled to [2, 8, 128, E]
    wo_nat = out_w.reshape(8, 128, Ex)
    wo_swp = np.concatenate(
        [wo_nat[:, 64:128, :], wo_nat[:, 0:64, :]], axis=1
    )
    wo2 = np.stack([wo_nat, wo_swp], axis=0).astype(np.float16)
    ob = out_b.astype(np.float16)

    in_maps = []
    for c in range(NCORES):
        b = c // 4
        h0 = 4 * (c % 4)
        xT = np.ascontiguousarray(x[:, b, :].T).astype(np.float16)
        wq, bq = [], []
        for g in range(2):
            ha = h0 + 2 * g
            cols = slice(ha * D, ha * D + 128)
            wq.append(np.stack(
                [in_proj_w[:, cols],
                 in_proj_w[:, Ex + ha * D: Ex + ha * D + 128],
                 in_proj_w[:, 2 * Ex + ha * D: 2 * Ex + ha * D + 128]], axis=0))
            bq.append(np.stack(
                [in_proj_b[cols],
                 in_proj_b[Ex + ha * D: Ex + ha * D + 128],
                 in_proj_b[2 * Ex + ha * D: 2 * Ex + ha * D + 128]], axis=0))
        in_maps.append({
            "xT": xT,
            "wqkv": np.stack(wq, axis=0).astype(np.float16),
            "bqkv": np.stack(bq, axis=0).astype(np.float32),
            "wo2": wo2,
            "ob": ob,
        })
    return in_maps


def assemble(results, T, Ex):
    R = T // 16
    yf = np.empty((B, T, Ex), np.float32)
    for c in range(NCORES):
        b = c // 4
        h0 = 4 * (c % 4)
        blk = results[c]["y"]  # [4, T//16, E]
        for i in range(4):
            h = h0 + i
            yf[b, h * R:(h + 1) * R, :] = blk[i]
    return yf


def kernel(x, in_proj_w, in_proj_b, out_w, out_b):
    T = x.shape[0]
    nc = _get_nc(T)
    in_maps = make_in_maps(x, in_proj_w, in_proj_b, out_w, out_b)
    last_err = None
    for _attempt in range(3):
        try:
            res = run_bass_kernel_spmd(nc, in_maps, core_ids=list(range(NCORES)))
            return assemble(res.results, T, x.shape[2])
        except Exception as e:  # transient NRT device wedge on first touch
            last_err = e
            import time as _time
            _time.sleep(5)
    raise last_err
